# revision 27
# baseline (speedup 1.0000x reference)
"""GCNConv Trainium2 kernel (8 NeuronCores, Bass/Tile).

out = relu( D^{-1/2} (A + I) D^{-1/2} (x W^T + b) )

Distribution: destination nodes (output rows) are sharded across 8 cores;
edges are partitioned by destination so the segment-sum is core-local. The
small weight/bias are replicated.

Device algorithm per core (dest rows R_m, |R_m| = N/8 = 6250):
  agg[n]  = sum_{e: dst=n} norm[e] * x[src[e]]   (self term = one more slot)
  out[n]  = relu( agg[n] @ W^T + P1[n] * b )     (P1[n] = sum norm over n)

Edge slots are packed per dest-group (<=128 dests per group, greedy-balanced
so the per-group chunk count -- a program constant shared by all 8 SPMD
cores -- carries minimal padding). The source-feature stream
xe[slot] = x[src[slot]] is laid out chunk-interleaved by the host and read
as bulk contiguous DMA (fp8 e3m4, 128B/row): bulk DMA is bandwidth-priced
while per-edge gather descriptors cost ~1.42ns/edge regardless of dtype
(256B elem granularity + the sub-512B descriptor penalty), which is why the
previous dma_gather design could never beat ~150us.

Per 128-slot chunk: one tensor_scalar builds the bf16 selection tile
S[slot, d] = norm[slot] * (dest[slot] == d) (split 6:2 across the Vector
and GpSimd engines -- GpSimd no longer generates gather descriptors so its
cycles are free), then PE accumulates aggT[feat, dest] += chunk^T S into
the group's [128,128] PSUM tile (fp8 stationary x bf16 moving, 1 row/cyc).
Group epilogue: Act copies PSUM->SBUF fp16, PE applies W plus the bias
outer-product, Act applies relu into the fp16 output slab; the host
un-permutes.

Synchronization is hand-rolled with counting semaphores at BLOCK (8-chunk)
granularity instead of the Tile framework's per-instruction waits: with
auto-sync, every S-build carried a ~75ns satisfied-wait instruction plus
~70ns issue on the build engine's sequencer, which serialized the whole
pipeline at ~120ns/chunk (104.9us) while no engine exceeded 59% busy.
Manual sems: builds run 3 blocks ahead of PE, gated by one wait per block;
PE waits twice per block for that block's builds; epilogues are deferred
one block so their cross-engine waits are pre-satisfied.

Numerics (validated against the fp64 reference on the actual inputs):
xe e3m4 + norm bf16 + fp16 agg/W/out gives rel err ~1.2e-2 (< 2e-2 gate);
e4m3 would fail (2.9e-2) and bf16-everything gives 2.5e-3.
"""

import math

import numpy as np

_N_CORES = 8
_P = 128  # partitions / feature dim / dest-group width
_PIECE = 32  # stream chunks per DMA piece
_SENT = 1000.0  # pad sentinel (matches no iota value)
_STORE_EVERY = 4  # groups per output store
_K = 8  # chunks per sync block
_BLAG = 15  # build run-ahead in blocks
_NSD = 104  # DVE selection-ring depth (>= _BLAG+1 blocks x ~6.5)
_NSP = 40  # GpSimd selection-ring depth
_NSA = 16  # Act selection-ring depth
_NXB = 6  # stream piece buffers
_ALOOK = 6  # Act build emission lookahead (blocks) past its epilogue stalls
_CD, _CP, _CA = 94, 273, 650  # per-build cost for the split (Act taxed:
# its serial epilogue chain quantizes stalls, so keep slack on it)
_CEPI = 199.0  # Act epilogue work per group (paired [128,256] copy+relu / 2)


# ---------------------------------------------------------------- host prep

def _host_prep(x, W, b, edge_weight, edge_index, n_cores):
    from ml_dtypes import bfloat16, float8_e3m4

    N, D = x.shape
    assert D == _P
    assert N % n_cores == 0
    nd = N // n_cores  # dest rows per core
    G = math.ceil(nd / _P)  # dest groups per core

    ei = np.asarray(edge_index)
    row = ei[0].astype(np.int64)
    col = ei[1].astype(np.int64)
    w = np.asarray(edge_weight, np.float64)

    # degree normalization (self-loop weight 1 included in the row sums)
    deg = 1.0 + np.bincount(row, weights=w, minlength=N)
    d_inv = 1.0 / np.sqrt(deg)
    norm = d_inv[row] * w * d_inv[col]
    norm_self = d_inv * d_inv
    p1 = (norm_self + np.bincount(row, weights=norm, minlength=N)).astype(np.float32)

    core_e = row // nd
    loc_e = row - core_e * nd

    # --- balanced dest->group assignment (per core) ---
    # Greedy: dests sorted by (self+edge) load, assigned to the least-loaded
    # group with capacity < 128, so per-group slot counts are even and the
    # cross-core max (the program constant) carries minimal padding.
    import heapq

    edeg = np.bincount(row, minlength=N).reshape(n_cores, nd)  # per-dest edge count
    # Planned per-group chunk caps summing to the lower bound
    # ceil(max_core_slots/128); the greedy below packs each core against
    # cap[g]*128 slot capacities (and <=128 dests/group), so the shared SPMD
    # chunk count carries near-zero padding.
    slots_m = edeg.sum(axis=1) + nd
    # +2 chunks of slack: at the exact lower bound the <=128-dests-per-group
    # constraint makes greedy LPT overflow by a few slots on the fullest core
    C_plan = int(-(-int(slots_m.max()) // _P)) + 2
    base, extra = divmod(C_plan, G)
    cap = np.full(G, base, np.int64)
    cap[:extra] += 1
    grp_of = np.zeros((n_cores, nd), np.int64)
    slot_of = np.zeros((n_cores, nd), np.int64)
    cnt_mg = np.zeros((n_cores, G), np.int64)  # slots (self+edges) per group
    for m in range(n_cores):
        load = edeg[m] + 1  # +1 self slot
        order = np.argsort(-load, kind="stable")
        ngrp = np.zeros(G, np.int64)
        # max-remaining-slack first (LPT against per-group slot capacity)
        heap = [(-cap[g] * _P, 0, g) for g in range(G)]
        heapq.heapify(heap)
        for dl in order:
            while True:
                negslack, nv, g = heapq.heappop(heap)
                if -negslack == cap[g] * _P - cnt_mg[m, g] and nv == ngrp[g] and ngrp[g] < _P:
                    break
            grp_of[m, dl] = g
            slot_of[m, dl] = ngrp[g]
            ngrp[g] += 1
            cnt_mg[m, g] += load[dl]
            if ngrp[g] < _P:
                heapq.heappush(heap, (cnt_mg[m, g] - cap[g] * _P, ngrp[g], g))
    pos_of = grp_of * _P + slot_of  # [M, nd] position in padded output space

    # final chunk caps: planned, bumped where a core overflowed
    cap = np.maximum(cap, -(-cnt_mg.max(axis=0) // _P))
    c0 = np.zeros(G + 1, np.int64)
    np.cumsum(cap, out=c0[1:])
    C = int(c0[G])  # total chunks

    # --- slot assignment ---
    # Group g's run occupies slots [c0[g]*128, (c0[g]+cap[g])*128); self slots
    # first (in dest-slot order), then edges, then sentinel pads.
    grp_e = grp_of[core_e, loc_e]
    dst_e = slot_of[core_e, loc_e]  # within-group dest index

    xe = np.zeros((n_cores, _P, C * _P), float8_e3m4)
    dest_arr = np.full((n_cores, _P, C), _SENT, np.float32)
    norm_arr = np.zeros((n_cores, _P, C), np.float32)
    p1_arr = np.zeros((n_cores, 1, G * _P), np.float16)

    x_f8 = np.asarray(x, np.float32).astype(float8_e3m4)

    def put(m, j, src_rows, dvals, nvals):
        ch = j // _P
        pr = j % _P
        xv = xe[m].reshape(_P, C, _P)
        xv[pr, ch, :] = x_f8[src_rows]
        dest_arr[m, pr, ch] = dvals.astype(np.float32)
        norm_arr[m, pr, ch] = nvals.astype(bfloat16).astype(np.float32)

    for m in range(n_cores):
        sel = core_e == m
        ge = grp_e[sel]
        de = dst_e[sel]
        ce = col[sel]
        ne = norm[sel]
        eorder = np.argsort(ge, kind="stable")
        ge = ge[eorder]
        de = de[eorder]
        ce = ce[eorder]
        ne = ne[eorder]
        # self slots: group-major, dest-slot order
        gself = grp_of[m]
        sself = slot_of[m]
        sorder = np.lexsort((sself, gself))
        gs = gself[sorder]
        rows_self = m * nd + sorder
        nself_g = np.bincount(gs, minlength=G)
        estart = np.zeros(G + 1, np.int64)
        np.cumsum(np.bincount(ge, minlength=G), out=estart[1:])
        j_self = c0[gs] * _P + np.arange(len(gs)) - np.repeat(
            np.concatenate(([0], np.cumsum(nself_g)[:-1])), nself_g
        )
        put(m, j_self, rows_self, sself[sorder].astype(np.float64),
            norm_self[m * nd + sorder])
        within = np.arange(len(ge)) - estart[ge]
        j_edge = c0[ge] * _P + nself_g[ge] + within
        put(m, j_edge, ce, de.astype(np.float64), ne)
        p1_arr[m, 0, pos_of[m]] = p1[m * nd : (m + 1) * nd]

    iota_bf = np.tile(np.arange(_P, dtype=np.float32), (_P, 1)).astype(bfloat16)
    CH0 = min(C, 96)
    hdr = np.zeros((n_cores, _P, 256 + 8 * CH0), np.uint8)
    hdr[:, :, :256] = iota_bf.view(np.uint8)[None]
    hdr[:, :, 256 : 256 + 4 * CH0] = dest_arr[:, :, :CH0].view(np.uint8)
    hdr[:, :, 256 + 4 * CH0 :] = norm_arr[:, :, :CH0].view(np.uint8)
    wT = np.ascontiguousarray(np.asarray(W, np.float32).T).astype(np.float16)
    bias = np.asarray(b, np.float32).reshape(1, _P).astype(np.float16)

    cfg = (N, nd, G, tuple(int(v) for v in cap), n_cores)
    in_maps = []
    for m in range(n_cores):
        in_maps.append(
            {
                "xe": xe[m],
                "dest": dest_arr[m],
                "enorm": norm_arr[m],
                "nnorm": -norm_arr[m],
                "hdr": hdr[m],
                "p1": p1_arr[m],
                "wT": wT,
                "bias": bias,
            }
        )
    return cfg, in_maps, pos_of


# ---------------------------------------------------------------- device program

def _build_program(cfg):
    from contextlib import ExitStack

    from concourse import bacc, mybir

    N, nd, G, cap, n_cores = cfg
    c0 = [0]
    for g in range(G):
        c0.append(c0[-1] + cap[g])
    C = c0[G]
    GP = G * _P
    f32 = mybir.dt.float32
    bf16 = mybir.dt.bfloat16
    fp16 = mybir.dt.float16
    fp8 = mybir.dt.float8e3

    NBLK = -(-C // _K)
    NP = -(-C // _PIECE)

    # per-chunk metadata
    grp = np.empty(C, np.int64)
    first = np.zeros(C, bool)
    last = np.zeros(C, bool)
    for g in range(G):
        grp[c0[g] : c0[g + 1]] = g
        first[c0[g]] = True
        last[c0[g + 1] - 1] = True
    # cost-weighted greedy split of S-builds across DVE / GpSimd / Act
    # (Act joins only past the header slice and carries its epilogue load
    # as a per-chunk handicap)
    eng_of = np.empty(C, np.int8)  # 0=DVE 1=Pool 2=Act
    acc = [0.0, 0.0, 0.0]
    costs = [float(_CD), float(_CP), float(_CA)]
    for c in range(C):
        acc[2] += _CEPI * G / C
        n_eng = 3 if c >= 96 else 2
        e = min(range(n_eng), key=lambda i: acc[i] + costs[i])
        eng_of[c] = e
        acc[e] += costs[e]
    on_dve = eng_of == 0
    on_pool = eng_of == 1
    on_act = eng_of == 2
    ring_idx = np.empty(C, np.int64)
    ring_idx[on_dve] = np.arange(on_dve.sum()) % _NSD
    ring_idx[on_pool] = np.arange(on_pool.sum()) % _NSP
    ring_idx[on_act] = np.arange(on_act.sum()) % _NSA
    ncum = np.cumsum(on_dve)  # DVE builds through chunk c (inclusive)
    pcum = np.cumsum(on_pool)
    acum = np.cumsum(on_act)
    blk_end = [min(C, (b + 1) * _K) - 1 for b in range(NBLK)]
    nd_cum = [int(ncum[e]) for e in blk_end]
    np_cum = [int(pcum[e]) for e in blk_end]
    na_cum = [int(acum[e]) for e in blk_end]
    act_blk = {}  # block -> list of (i, c) Act builds
    ai = 0
    for c in range(C):
        if on_act[c]:
            act_blk.setdefault(c // _K, []).append((ai, c))
            ai += 1
    piece = [c // _PIECE for c in range(C)]
    grp_end_blk = [(c0[g + 1] - 1) // _K for g in range(G)]
    n_stores = -(-G // _STORE_EVERY)

    nc = bacc.Bacc(
        "TRN2",
        target_bir_lowering=False,
        debug=False,
        enable_asserts=False,
        num_devices=n_cores,
    )
    xe_d = nc.dram_tensor("xe", [_P, C * _P], fp8, kind="ExternalInput").ap()
    dest_d = nc.dram_tensor("dest", [_P, C], f32, kind="ExternalInput").ap()
    norm_d = nc.dram_tensor("enorm", [_P, C], f32, kind="ExternalInput").ap()
    nnorm_d = nc.dram_tensor("nnorm", [_P, C], f32, kind="ExternalInput").ap()
    p1_d = nc.dram_tensor("p1", [1, GP], fp16, kind="ExternalInput").ap()
    wT_d = nc.dram_tensor("wT", [_P, _P], fp16, kind="ExternalInput").ap()
    b_d = nc.dram_tensor("bias", [1, _P], fp16, kind="ExternalInput").ap()
    u8 = mybir.dt.uint8
    CH0 = min(C, 96)
    hdr_d = nc.dram_tensor("hdr", [_P, 256 + 8 * CH0], u8, kind="ExternalInput").ap()
    out_d = nc.dram_tensor("outT", [_P, GP], fp16, kind="ExternalOutput").ap()

    with ExitStack() as ctx:
        sem = {}
        for s in (
            "sd", "sp", "sa", "tmp", "mmblk", "aggT", "ps2", "relu",
            "cdma", "cdmb", "cdmn", "wdma", "outdma",
        ):
            sem[s] = ctx.enter_context(nc.semaphore(f"s_{s}"))
        # one sem per stream buffer slot: a DMA's +16 arrives as 16
        # independent +1s, so concurrent transfers sharing a sem can
        # interleave and satisfy intermediate waits spuriously
        xe_sems = [
            ctx.enter_context(nc.semaphore(f"s_xe{j}")) for j in range(_NXB)
        ]

        def sb(name, shape, dt):
            return ctx.enter_context(nc.sbuf_tensor(name, shape, dt))

        dest_t = sb("dest_t", [_P, C], f32)
        norm_t = sb("norm_t", [_P, C], f32)
        nnorm_t = sb("nnorm_t", [_P, C], f32)
        hdr_t = sb("hdr_t", [_P, 256 + 8 * CH0], u8)
        iota_t = hdr_t[:, :256].bitcast(bf16)
        destA = hdr_t[:, 256 : 256 + 4 * CH0].bitcast(f32)
        normA = hdr_t[:, 256 + 4 * CH0 :].bitcast(f32)

        def dcol(c):
            return destA[:, c : c + 1] if c < CH0 else dest_t[:, c : c + 1]

        def ncol(c):
            return normA[:, c : c + 1] if c < CH0 else norm_t[:, c : c + 1]
        wT_t = sb("wT_t", [_P, _P], fp16)
        b_t = sb("b_t", [1, _P], fp16)
        p1_t = sb("p1_t", [1, GP], fp16)
        out_t = sb("out_t", [_P, GP], fp16)
        sd = [sb(f"sd{i}", [_P, _P], bf16) for i in range(_NSD)]
        sp = [sb(f"sp{i}", [_P, _P], bf16) for i in range(_NSP)]
        sa = [sb(f"sa{i}", [_P, _P], bf16) for i in range(_NSA)]
        stmp = [sb(f"stmp{i}", [_P, _P], bf16) for i in range(8)]
        xep = [sb(f"xep{i}", [_P, _PIECE * _P], fp8) for i in range(_NXB)]
        # groups are processed in pairs: group g lives in slice (g%2) of
        # pair-tensor (g//2)%2, so one [128,256] Act op covers two groups.
        # (Quads -- 4 groups per 2KB PSUM bank -- corrupt the last slice:
        # overlapping accumulation windows within one bank.)
        aggT = [sb(f"aggT{i}", [_P, 2 * _P], fp16) for i in range(2)]
        ps1 = [
            ctx.enter_context(nc.psum_tensor(f"ps1_{i}", [_P, 2 * _P], f32))
            for i in range(4)
        ]
        ps2 = [
            ctx.enter_context(nc.psum_tensor(f"ps2_{i}", [_P, 2 * _P], f32))
            for i in range(2)
        ]

        with nc.Block() as block:

            @block.sync
            def _(sync):
                # scalar arrays in two slices so the first builds start early

                # piece p covers blocks [4p, 4p+4) exactly (_PIECE == 4*_K),
                # so buffer reuse is gated on the block-progress sem
                for p in range(NP):
                    if p >= _NXB:
                        sync.wait_ge(sem["mmblk"], min(NBLK, 4 * (p - _NXB) + 4))
                    lo = p * _PIECE
                    wdt = min(C, lo + _PIECE) - lo
                    sync.dma_start(
                        xep[p % _NXB][:, : wdt * _P],
                        xe_d[:, lo * _P : (lo + wdt) * _P],
                    ).then_inc(xe_sems[p % _NXB], 16)
                    if p == 0:
                        sync.dma_start(dest_t[:, CH0:], dest_d[:, CH0:]).then_inc(sem["cdmb"], 16)
                        sync.dma_start(norm_t[:, CH0:], norm_d[:, CH0:]).then_inc(sem["cdmb"], 16)
                        sync.dma_start(nnorm_t[:, :], nnorm_d).then_inc(sem["cdmn"], 16)

                    if p == 1:
                        sync.dma_start(wT_t[:, :], wT_d).then_inc(sem["wdma"], 16)
                        sync.dma_start(b_t[:, :], b_d).then_inc(sem["wdma"], 16)
                        sync.dma_start(p1_t[:, :], p1_d).then_inc(sem["wdma"], 16)
                sync.wait_ge(sem["outdma"], 16 * n_stores)

            @block.scalar
            def _(scalar):
                scalar.dma_start(hdr_t[:, :], hdr_d).then_inc(sem["cdma"], 16)
                awaited = [False]
                emitted_b = [0]
                ntmp = [0]
                pend = []  # deferred T2s: (i, chunk, block)

                def act_T2(i, c):
                    scalar.wait_ge(sem["tmp"], i + 1)
                    scalar.activation(
                        out=sa[ring_idx[c]][:, :],
                        in_=stmp[i % 8][:, :],
                        func=mybir.ActivationFunctionType.Relu,
                        bias=norm_t[:, c : c + 1],
                        scale=nnorm_t[:, c : c + 1],
                    ).then_inc(sem["sa"], 1)

                def drain_pend(blk_lt):
                    """Emit deferred T2s for builds in blocks < blk_lt."""
                    while pend and pend[0][2] < blk_lt:
                        i, c, _ = pend.pop(0)
                        act_T2(i, c)

                def emit_builds_through(bmax):
                    """Act's S-builds for blocks < bmax: tmp = |dest - iota|,
                    S = relu(norm - norm*tmp) -- norm at tmp==0, else 0.
                    Engine ops are pipelined with no forwarding, so the T2
                    is deferred until its T1's sem edge is stale."""
                    for b in range(emitted_b[0], min(bmax, NBLK)):
                        for i, c in act_blk.get(b, []):
                            if not awaited[0]:
                                scalar.wait_ge(sem["cdmb"], 32)
                                scalar.wait_ge(sem["cdmn"], 16)
                                awaited[0] = True
                            if b >= _BLAG + 1:
                                scalar.wait_ge(sem["mmblk"], b - _BLAG)
                            while len(pend) > 7:
                                # the stmp-ring WAR below needs T2(i-8)
                                # emitted; cap the deferral depth
                                i2, c2, _ = pend.pop(0)
                                act_T2(i2, c2)
                            if i >= 8:
                                # stmp ring WAR vs T2(i-8)'s read
                                scalar.wait_ge(sem["sa"], i - 7)
                            scalar.activation(
                                out=stmp[i % 8][:, :],
                                in_=iota_t[:, :],
                                func=mybir.ActivationFunctionType.Abs,
                                bias=dest_t[:, c : c + 1],
                                scale=-1.0,
                            ).then_inc(sem["tmp"], 1)
                            ntmp[0] = i + 1
                            pend.append((i, c, b))
                    emitted_b[0] = max(emitted_b[0], min(bmax, NBLK))

                stored = 0
                NPAIR = -(-G // 2)
                for p in range(NPAIR + 1):
                    if p < NPAIR:
                        glo = 2 * p
                        ghi = min(G - 1, glo + 1)  # last pair may be single
                        wd = (ghi - glo + 1) * _P
                        emit_builds_through(grp_end_blk[ghi] + 2 + _ALOOK)
                        # PE consuming block V-1 needs this engine's deferred
                        # T2s for blocks < V: drain them before blocking on PE
                        drain_pend(grp_end_blk[ghi] + 1)
                        # the pair's PSUM is final once the block holding the
                        # later group's last chunk completes (one sem update
                        # per matmul: the compiler rejects a second .then_inc)
                        scalar.wait_ge(sem["mmblk"], grp_end_blk[ghi] + 1)
                        scalar.copy(
                            out=aggT[p % 2][:, :wd], in_=ps1[p % 4][:, :wd]
                        ).then_inc(sem["aggT"], 1)
                    if p >= 1:
                        q = p - 1
                        jlo = 2 * q
                        jhi = min(G - 1, jlo + 1)
                        wd = (jhi - jlo + 1) * _P
                        scalar.wait_ge(sem["ps2"], jhi + 1)
                        scalar.activation(
                            out=out_t[:, jlo * _P : jlo * _P + wd],
                            in_=ps2[q % 2][:, :wd],
                            func=mybir.ActivationFunctionType.Relu,
                        ).then_inc(sem["relu"], 1)
                        # deferred store: flush groups whose relu finished a
                        # pair ago, so the read-after-write sem edge on out_t
                        # is already stale when the DMA issues
                        if q >= 1 and (jlo % _STORE_EVERY == 0) and stored < jlo:
                            scalar.wait_ge(sem["relu"], q)
                            scalar.dma_start(
                                out_d[:, stored * _P : jlo * _P],
                                out_t[:, stored * _P : jlo * _P],
                            ).then_inc(sem["outdma"], 16)
                            stored = jlo
                        if jhi + 1 == G:
                            scalar.wait_ge(sem["relu"], q + 1)
                            scalar.dma_start(
                                out_d[:, stored * _P : (jhi + 1) * _P],
                                out_t[:, stored * _P : (jhi + 1) * _P],
                            ).then_inc(sem["outdma"], 16)
                            stored = jhi + 1

            @block.vector
            def _(vector):
                vector.wait_ge(sem["cdma"], 16)
                vwaitb = [False]
                for b in range(NBLK):
                    if b >= _BLAG + 1:
                        vector.wait_ge(sem["mmblk"], b - _BLAG)
                    for c in range(b * _K, min(C, (b + 1) * _K)):
                        if c >= 96 and not vwaitb[0]:
                            vector.wait_ge(sem["cdmb"], 32)
                            vwaitb[0] = True
                        if on_dve[c]:
                            vector.tensor_scalar(
                                out=sd[ring_idx[c]][:, :],
                                in0=iota_t[:, :],
                                scalar1=dcol(c),
                                scalar2=ncol(c),
                                op0=mybir.AluOpType.is_equal,
                                op1=mybir.AluOpType.mult,
                            ).then_inc(sem["sd"], 1)

            @block.gpsimd
            def _(gpsimd):
                gpsimd.wait_ge(sem["cdma"], 16)
                pwaitb = [False]
                for b in range(NBLK):
                    if b >= _BLAG + 1:
                        gpsimd.wait_ge(sem["mmblk"], b - _BLAG)
                    for c in range(b * _K, min(C, (b + 1) * _K)):
                        if c >= 96 and not pwaitb[0]:
                            gpsimd.wait_ge(sem["cdmb"], 32)
                            pwaitb[0] = True
                        if on_pool[c]:
                            gpsimd.tensor_scalar(
                                out=sp[ring_idx[c]][:, :],
                                in0=iota_t[:, :],
                                scalar1=dcol(c),
                                scalar2=ncol(c),
                                op0=mybir.AluOpType.is_equal,
                                op1=mybir.AluOpType.mult,
                            ).then_inc(sem["sp"], 1)

            @block.tensor
            def _(tensor):
                epi = 0  # next group awaiting epilogue

                def epilogue(g):
                    # sem["aggT"]/sem["relu"] count PAIRS (2 groups each)
                    tensor.wait_ge(sem["aggT"], g // 2 + 1)
                    if g == 0:
                        tensor.wait_ge(sem["wdma"], 48)
                    if g >= 4:
                        tensor.wait_ge(sem["relu"], g // 2 - 1)
                    sl = (g % 2) * _P
                    tensor.matmul(
                        ps2[(g // 2) % 2][:, sl : sl + _P],
                        b_t[:, :],
                        p1_t[0:1, g * _P : (g + 1) * _P],
                        start=True,
                        stop=False,
                    )
                    tensor.matmul(
                        ps2[(g // 2) % 2][:, sl : sl + _P],
                        wT_t[:, :],
                        aggT[(g // 2) % 2][:, sl : sl + _P],
                        start=False,
                        stop=True,
                    ).then_inc(sem["ps2"], 1)

                for b in range(NBLK):
                    tensor.wait_ge(sem["sd"], nd_cum[b])
                    if b == 0 or np_cum[b] > np_cum[b - 1]:
                        tensor.wait_ge(sem["sp"], np_cum[b])
                    if na_cum[b] > (0 if b == 0 else na_cum[b - 1]):
                        tensor.wait_ge(sem["sa"], na_cum[b])
                    for c in range(b * _K, min(C, (b + 1) * _K)):
                        if c % _PIECE == 0:
                            tensor.wait_ge(
                                xe_sems[piece[c] % _NXB],
                                16 * (piece[c] // _NXB + 1),
                            )
                        g = grp[c]
                        if first[c] and g >= 8:
                            # ps1 slice reuse: pair copy g//2-4 must be done
                            tensor.wait_ge(sem["aggT"], g // 2 - 3)
                        S = (sd, sp, sa)[eng_of[c]][ring_idx[c]]
                        off = (c % _PIECE) * _P
                        gsl = (g % 2) * _P
                        mm = tensor.matmul(
                            ps1[(g // 2) % 4][:, gsl : gsl + _P],
                            xep[piece[c] % _NXB][:, off : off + _P],
                            S[:, :],
                            start=bool(first[c]),
                            stop=bool(last[c]),
                        )
                        if c == blk_end[b]:
                            mm.then_inc(sem["mmblk"], 1)
                    # an epilogue is emitted 3 blocks after its PAIR's later
                    # group finishes, so the Act pair-copy it waits on has
                    # long completed and the wait never stalls the PE SEQ
                    while epi < G and grp_end_blk[min(G - 1, (epi // 2) * 2 + 1)] <= b - 3:
                        epilogue(epi)
                        epi += 1
                while epi < G:
                    epilogue(epi)
                    epi += 1

        nc.compile()
    return nc


_program_cache: dict = {}


def _get_program(cfg):
    if cfg not in _program_cache:
        _program_cache[cfg] = _build_program(cfg)
    return _program_cache[cfg]


# ---------------------------------------------------------------- entry points

def run(inputs: dict, trace: bool = False, n_cores: int = _N_CORES):
    """Run the kernel; returns (full_output, BassKernelResults)."""
    from concourse import bass_utils

    cfg, in_maps, pos_of = _host_prep(
        inputs["x"],
        inputs["W"],
        inputs["b"],
        inputs["edge_weight"],
        inputs["edge_index"],
        n_cores,
    )
    nc = _get_program(cfg)
    try:
        res = bass_utils.run_bass_kernel_spmd(
            nc, in_maps, core_ids=list(range(n_cores)), trace=trace
        )
    except Exception:
        # the axon-tunneled device occasionally reports a transient
        # NRT_EXEC_UNIT_UNRECOVERABLE right after a crashed/heavy prior run;
        # reconnect the backend and retry once before giving up
        import time as _time

        import jax as _jax

        _time.sleep(5.0)
        try:
            _jax.clear_backends()
        except Exception:
            pass
        res = bass_utils.run_bass_kernel_spmd(
            nc, in_maps, core_ids=list(range(n_cores)), trace=trace
        )
    N, nd = cfg[0], cfg[1]
    out = np.empty((N, _P), np.float32)
    for m in range(n_cores):
        slab = np.asarray(res.results[m]["outT"]).astype(np.float32).T  # [GP, 128]
        out[m * nd : (m + 1) * nd, :] = slab[pos_of[m]]
    return out, res


def kernel(**inputs) -> np.ndarray:
    out, _ = run(inputs, trace=False)
    return out


# revision 28
# speedup vs baseline: 1.7227x; 1.7227x over previous
"""GCNConv Trainium2 kernel (8 NeuronCores, Bass/Tile).

out = relu( D^{-1/2} (A + I) D^{-1/2} (x W^T + b) )

Distribution: destination nodes (output rows) are sharded across 8 cores;
edges are partitioned by destination so the segment-sum is core-local. The
small weight/bias are replicated.

Device algorithm per core (dest rows R_m, |R_m| = N/8 = 6250):
  agg[n]  = sum_{e: dst=n} norm[e] * x[src[e]]   (self term = one more slot)
  out[n]  = relu( agg[n] @ W^T + P1[n] * b )     (P1[n] = sum norm over n)

Edge slots are packed per dest-group (<=128 dests per group, greedy-balanced
so the per-group chunk count -- a program constant shared by all 8 SPMD
cores -- carries minimal padding). The source-feature stream
xe[slot] = x[src[slot]] is laid out chunk-interleaved by the host and read
as bulk contiguous DMA (fp8 e3m4, 128B/row): bulk DMA is bandwidth-priced
while per-edge gather descriptors cost ~1.42ns/edge regardless of dtype
(256B elem granularity + the sub-512B descriptor penalty), which is why the
previous dma_gather design could never beat ~150us.

Per 128-slot chunk: one tensor_scalar builds the bf16 selection tile
S[slot, d] = norm[slot] * (dest[slot] == d) (split 6:2 across the Vector
and GpSimd engines -- GpSimd no longer generates gather descriptors so its
cycles are free), then PE accumulates aggT[feat, dest] += chunk^T S into
the group's [128,128] PSUM tile (fp8 stationary x bf16 moving, 1 row/cyc).
Group epilogue: Act copies PSUM->SBUF fp16, PE applies W plus the bias
outer-product, Act applies relu into the fp16 output slab; the host
un-permutes.

Synchronization is hand-rolled with counting semaphores at BLOCK (8-chunk)
granularity instead of the Tile framework's per-instruction waits: with
auto-sync, every S-build carried a ~75ns satisfied-wait instruction plus
~70ns issue on the build engine's sequencer, which serialized the whole
pipeline at ~120ns/chunk (104.9us) while no engine exceeded 59% busy.
Manual sems: builds run 3 blocks ahead of PE, gated by one wait per block;
PE waits twice per block for that block's builds; epilogues are deferred
one block so their cross-engine waits are pre-satisfied.

Numerics (validated against the fp64 reference on the actual inputs):
xe e3m4 + norm bf16 + fp16 agg/W/out gives rel err ~1.2e-2 (< 2e-2 gate);
e4m3 would fail (2.9e-2) and bf16-everything gives 2.5e-3.
"""

import math

import numpy as np

_N_CORES = 8
_P = 128  # partitions / feature dim / dest-group width
_PIECE = 32  # stream chunks per DMA piece
_SENT = 1000.0  # pad sentinel (matches no iota value)
_STORE_EVERY = 4  # groups per output store
_K = 8  # chunks per sync block
_BLAG = 15  # build run-ahead in blocks
_NSD = 104  # DVE selection-ring depth (>= _BLAG+1 blocks x ~6.5)
_NSP = 40  # GpSimd selection-ring depth
_NSA = 16  # Act selection-ring depth
_NXB = 6  # stream piece buffers
_ALOOK = 6  # Act build emission lookahead (blocks) past its epilogue stalls
_CD, _CP, _CA = 94, 273, 900  # per-build cost for the split (Act taxed:
# its serial epilogue chain quantizes stalls, so keep slack on it)
_CEPI = 199.0  # Act epilogue work per group (paired [128,256] copy+relu / 2)


# ---------------------------------------------------------------- host prep

def _host_prep(x, W, b, edge_weight, edge_index, n_cores):
    from ml_dtypes import bfloat16, float8_e3m4

    N, D = x.shape
    assert D == _P
    assert N % n_cores == 0
    nd = N // n_cores  # dest rows per core
    G = math.ceil(nd / _P)  # dest groups per core

    ei = np.asarray(edge_index)
    row = ei[0].astype(np.int64)
    col = ei[1].astype(np.int64)
    w = np.asarray(edge_weight, np.float64)

    # degree normalization (self-loop weight 1 included in the row sums)
    deg = 1.0 + np.bincount(row, weights=w, minlength=N)
    d_inv = 1.0 / np.sqrt(deg)
    norm = d_inv[row] * w * d_inv[col]
    norm_self = d_inv * d_inv
    p1 = (norm_self + np.bincount(row, weights=norm, minlength=N)).astype(np.float32)

    core_e = row // nd
    loc_e = row - core_e * nd

    # --- balanced dest->group assignment (per core) ---
    # Greedy: dests sorted by (self+edge) load, assigned to the least-loaded
    # group with capacity < 128, so per-group slot counts are even and the
    # cross-core max (the program constant) carries minimal padding.
    import heapq

    edeg = np.bincount(row, minlength=N).reshape(n_cores, nd)  # per-dest edge count
    # Planned per-group chunk caps summing to the lower bound
    # ceil(max_core_slots/128); the greedy below packs each core against
    # cap[g]*128 slot capacities (and <=128 dests/group), so the shared SPMD
    # chunk count carries near-zero padding.
    slots_m = edeg.sum(axis=1) + nd
    # +2 chunks of slack: at the exact lower bound the <=128-dests-per-group
    # constraint makes greedy LPT overflow by a few slots on the fullest core
    C_plan = int(-(-int(slots_m.max()) // _P)) + 2
    base, extra = divmod(C_plan, G)
    cap = np.full(G, base, np.int64)
    cap[:extra] += 1
    grp_of = np.zeros((n_cores, nd), np.int64)
    slot_of = np.zeros((n_cores, nd), np.int64)
    cnt_mg = np.zeros((n_cores, G), np.int64)  # slots (self+edges) per group
    for m in range(n_cores):
        load = edeg[m] + 1  # +1 self slot
        order = np.argsort(-load, kind="stable")
        ngrp = np.zeros(G, np.int64)
        # max-remaining-slack first (LPT against per-group slot capacity)
        heap = [(-cap[g] * _P, 0, g) for g in range(G)]
        heapq.heapify(heap)
        for dl in order:
            while True:
                negslack, nv, g = heapq.heappop(heap)
                if -negslack == cap[g] * _P - cnt_mg[m, g] and nv == ngrp[g] and ngrp[g] < _P:
                    break
            grp_of[m, dl] = g
            slot_of[m, dl] = ngrp[g]
            ngrp[g] += 1
            cnt_mg[m, g] += load[dl]
            if ngrp[g] < _P:
                heapq.heappush(heap, (cnt_mg[m, g] - cap[g] * _P, ngrp[g], g))
    pos_of = grp_of * _P + slot_of  # [M, nd] position in padded output space

    # final chunk caps: planned, bumped where a core overflowed
    cap = np.maximum(cap, -(-cnt_mg.max(axis=0) // _P))
    c0 = np.zeros(G + 1, np.int64)
    np.cumsum(cap, out=c0[1:])
    C = int(c0[G])  # total chunks

    # --- slot assignment ---
    # Group g's run occupies slots [c0[g]*128, (c0[g]+cap[g])*128); self slots
    # first (in dest-slot order), then edges, then sentinel pads.
    grp_e = grp_of[core_e, loc_e]
    dst_e = slot_of[core_e, loc_e]  # within-group dest index

    xe = np.zeros((n_cores, _P, C * _P), float8_e3m4)
    dest_arr = np.full((n_cores, _P, C), _SENT, np.float32)
    norm_arr = np.zeros((n_cores, _P, C), np.float32)
    p1_arr = np.zeros((n_cores, 1, G * _P), np.float16)

    x_f8 = np.asarray(x, np.float32).astype(float8_e3m4)

    def put(m, j, src_rows, dvals, nvals):
        ch = j // _P
        pr = j % _P
        xv = xe[m].reshape(_P, C, _P)
        xv[pr, ch, :] = x_f8[src_rows]
        dest_arr[m, pr, ch] = dvals.astype(np.float32)
        norm_arr[m, pr, ch] = nvals.astype(bfloat16).astype(np.float32)

    for m in range(n_cores):
        sel = core_e == m
        ge = grp_e[sel]
        de = dst_e[sel]
        ce = col[sel]
        ne = norm[sel]
        eorder = np.argsort(ge, kind="stable")
        ge = ge[eorder]
        de = de[eorder]
        ce = ce[eorder]
        ne = ne[eorder]
        # self slots: group-major, dest-slot order
        gself = grp_of[m]
        sself = slot_of[m]
        sorder = np.lexsort((sself, gself))
        gs = gself[sorder]
        rows_self = m * nd + sorder
        nself_g = np.bincount(gs, minlength=G)
        estart = np.zeros(G + 1, np.int64)
        np.cumsum(np.bincount(ge, minlength=G), out=estart[1:])
        j_self = c0[gs] * _P + np.arange(len(gs)) - np.repeat(
            np.concatenate(([0], np.cumsum(nself_g)[:-1])), nself_g
        )
        put(m, j_self, rows_self, sself[sorder].astype(np.float64),
            norm_self[m * nd + sorder])
        within = np.arange(len(ge)) - estart[ge]
        j_edge = c0[ge] * _P + nself_g[ge] + within
        put(m, j_edge, ce, de.astype(np.float64), ne)
        p1_arr[m, 0, pos_of[m]] = p1[m * nd : (m + 1) * nd]

    iota_bf = np.tile(np.arange(_P, dtype=np.float32), (_P, 1)).astype(bfloat16)
    CH0 = min(C, 96)
    hdr = np.zeros((n_cores, _P, 256 + 8 * CH0), np.uint8)
    hdr[:, :, :256] = iota_bf.view(np.uint8)[None]
    hdr[:, :, 256 : 256 + 4 * CH0] = dest_arr[:, :, :CH0].view(np.uint8)
    hdr[:, :, 256 + 4 * CH0 :] = norm_arr[:, :, :CH0].view(np.uint8)
    wT = np.ascontiguousarray(np.asarray(W, np.float32).T).astype(np.float16)
    bias = np.asarray(b, np.float32).reshape(1, _P).astype(np.float16)

    cfg = (N, nd, G, tuple(int(v) for v in cap), n_cores)
    in_maps = []
    for m in range(n_cores):
        in_maps.append(
            {
                "xe": xe[m],
                "dest": dest_arr[m],
                "enorm": norm_arr[m],
                "hdr": hdr[m],
                "p1": p1_arr[m],
                "wT": wT,
                "bias": bias,
            }
        )
    return cfg, in_maps, pos_of


# ---------------------------------------------------------------- device program

def _build_program(cfg):
    from contextlib import ExitStack

    from concourse import bacc, mybir

    N, nd, G, cap, n_cores = cfg
    c0 = [0]
    for g in range(G):
        c0.append(c0[-1] + cap[g])
    C = c0[G]
    GP = G * _P
    f32 = mybir.dt.float32
    bf16 = mybir.dt.bfloat16
    fp16 = mybir.dt.float16
    fp8 = mybir.dt.float8e3

    NBLK = -(-C // _K)
    NP = -(-C // _PIECE)

    # per-chunk metadata
    grp = np.empty(C, np.int64)
    first = np.zeros(C, bool)
    last = np.zeros(C, bool)
    for g in range(G):
        grp[c0[g] : c0[g + 1]] = g
        first[c0[g]] = True
        last[c0[g + 1] - 1] = True
    # cost-weighted greedy split of S-builds across DVE / GpSimd
    eng_of = np.empty(C, np.int8)  # 0=DVE 1=Pool
    acc = [0.0, 0.0]
    costs = [float(_CD), float(_CP)]
    for c in range(C):
        e = min(range(2), key=lambda i: acc[i] + costs[i])
        eng_of[c] = e
        acc[e] += costs[e]
    on_dve = eng_of == 0
    on_pool = eng_of == 1
    ring_idx = np.empty(C, np.int64)
    ring_idx[on_dve] = np.arange(on_dve.sum()) % _NSD
    ring_idx[on_pool] = np.arange(on_pool.sum()) % _NSP
    ncum = np.cumsum(on_dve)  # DVE builds through chunk c (inclusive)
    pcum = np.cumsum(on_pool)
    blk_end = [min(C, (b + 1) * _K) - 1 for b in range(NBLK)]
    nd_cum = [int(ncum[e]) for e in blk_end]
    np_cum = [int(pcum[e]) for e in blk_end]
    piece = [c // _PIECE for c in range(C)]
    grp_end_blk = [(c0[g + 1] - 1) // _K for g in range(G)]
    n_stores = -(-G // _STORE_EVERY)

    nc = bacc.Bacc(
        "TRN2",
        target_bir_lowering=False,
        debug=False,
        enable_asserts=False,
        num_devices=n_cores,
    )
    xe_d = nc.dram_tensor("xe", [_P, C * _P], fp8, kind="ExternalInput").ap()
    dest_d = nc.dram_tensor("dest", [_P, C], f32, kind="ExternalInput").ap()
    norm_d = nc.dram_tensor("enorm", [_P, C], f32, kind="ExternalInput").ap()
    p1_d = nc.dram_tensor("p1", [1, GP], fp16, kind="ExternalInput").ap()
    wT_d = nc.dram_tensor("wT", [_P, _P], fp16, kind="ExternalInput").ap()
    b_d = nc.dram_tensor("bias", [1, _P], fp16, kind="ExternalInput").ap()
    u8 = mybir.dt.uint8
    CH0 = min(C, 96)
    hdr_d = nc.dram_tensor("hdr", [_P, 256 + 8 * CH0], u8, kind="ExternalInput").ap()
    out_d = nc.dram_tensor("outT", [_P, GP], fp16, kind="ExternalOutput").ap()

    with ExitStack() as ctx:
        sem = {}
        for s in (
            "sd", "sp", "mmblk", "aggT", "ps2", "relu",
            "cdma", "cdmb", "wdma", "outdma",
        ):
            sem[s] = ctx.enter_context(nc.semaphore(f"s_{s}"))
        # one sem per stream buffer slot: a DMA's +16 arrives as 16
        # independent +1s, so concurrent transfers sharing a sem can
        # interleave and satisfy intermediate waits spuriously
        xe_sems = [
            ctx.enter_context(nc.semaphore(f"s_xe{j}")) for j in range(_NXB)
        ]

        def sb(name, shape, dt):
            return ctx.enter_context(nc.sbuf_tensor(name, shape, dt))

        dest_t = sb("dest_t", [_P, C], f32)
        norm_t = sb("norm_t", [_P, C], f32)
        hdr_t = sb("hdr_t", [_P, 256 + 8 * CH0], u8)
        iota_t = hdr_t[:, :256].bitcast(bf16)
        destA = hdr_t[:, 256 : 256 + 4 * CH0].bitcast(f32)
        normA = hdr_t[:, 256 + 4 * CH0 :].bitcast(f32)

        def dcol(c):
            return destA[:, c : c + 1] if c < CH0 else dest_t[:, c : c + 1]

        def ncol(c):
            return normA[:, c : c + 1] if c < CH0 else norm_t[:, c : c + 1]
        wT_t = sb("wT_t", [_P, _P], fp16)
        b_t = sb("b_t", [1, _P], fp16)
        p1_t = sb("p1_t", [1, GP], fp16)
        out_t = sb("out_t", [_P, GP], fp16)
        sd = [sb(f"sd{i}", [_P, _P], bf16) for i in range(_NSD)]
        sp = [sb(f"sp{i}", [_P, _P], bf16) for i in range(_NSP)]
        xep = [sb(f"xep{i}", [_P, _PIECE * _P], fp8) for i in range(_NXB)]
        # groups are processed in pairs: group g lives in slice (g%2) of
        # pair-tensor (g//2)%2, so one [128,256] Act op covers two groups.
        # (Quads -- 4 groups per 2KB PSUM bank -- corrupt the last slice:
        # overlapping accumulation windows within one bank.)
        aggT = [sb(f"aggT{i}", [_P, 2 * _P], fp16) for i in range(2)]
        ps1 = [
            ctx.enter_context(nc.psum_tensor(f"ps1_{i}", [_P, 2 * _P], f32))
            for i in range(4)
        ]
        ps2 = [
            ctx.enter_context(nc.psum_tensor(f"ps2_{i}", [_P, 2 * _P], f32))
            for i in range(2)
        ]

        with nc.Block() as block:

            @block.sync
            def _(sync):
                # scalar arrays in two slices so the first builds start early

                # piece p covers blocks [4p, 4p+4) exactly (_PIECE == 4*_K),
                # so buffer reuse is gated on the block-progress sem
                for p in range(NP):
                    if p >= _NXB:
                        sync.wait_ge(sem["mmblk"], min(NBLK, 4 * (p - _NXB) + 4))
                    lo = p * _PIECE
                    wdt = min(C, lo + _PIECE) - lo
                    sync.dma_start(
                        xep[p % _NXB][:, : wdt * _P],
                        xe_d[:, lo * _P : (lo + wdt) * _P],
                    ).then_inc(xe_sems[p % _NXB], 16)
                    if p == 0:
                        sync.dma_start(dest_t[:, CH0:], dest_d[:, CH0:]).then_inc(sem["cdmb"], 16)
                        sync.dma_start(norm_t[:, CH0:], norm_d[:, CH0:]).then_inc(sem["cdmb"], 16)

                    if p == 1:
                        sync.dma_start(wT_t[:, :], wT_d).then_inc(sem["wdma"], 16)
                        sync.dma_start(b_t[:, :], b_d).then_inc(sem["wdma"], 16)
                        sync.dma_start(p1_t[:, :], p1_d).then_inc(sem["wdma"], 16)
                sync.wait_ge(sem["outdma"], 16 * n_stores)

            @block.scalar
            def _(scalar):
                scalar.dma_start(hdr_t[:, :], hdr_d).then_inc(sem["cdma"], 16)

                stored = 0
                NPAIR = -(-G // 2)
                for p in range(NPAIR + 1):
                    if p < NPAIR:
                        glo = 2 * p
                        ghi = min(G - 1, glo + 1)  # last pair may be single
                        wd = (ghi - glo + 1) * _P
                        # the pair's PSUM is final once the block holding the
                        # later group's last chunk completes (one sem update
                        # per matmul: the compiler rejects a second .then_inc)
                        scalar.wait_ge(sem["mmblk"], grp_end_blk[ghi] + 1)
                        scalar.copy(
                            out=aggT[p % 2][:, :wd], in_=ps1[p % 4][:, :wd]
                        ).then_inc(sem["aggT"], 1)
                    if p >= 1:
                        q = p - 1
                        jlo = 2 * q
                        jhi = min(G - 1, jlo + 1)
                        wd = (jhi - jlo + 1) * _P
                        scalar.wait_ge(sem["ps2"], jhi + 1)
                        scalar.activation(
                            out=out_t[:, jlo * _P : jlo * _P + wd],
                            in_=ps2[q % 2][:, :wd],
                            func=mybir.ActivationFunctionType.Relu,
                        ).then_inc(sem["relu"], 1)
                        # deferred store: flush groups whose relu finished a
                        # pair ago, so the read-after-write sem edge on out_t
                        # is already stale when the DMA issues
                        if q >= 1 and (jlo % _STORE_EVERY == 0) and stored < jlo:
                            scalar.wait_ge(sem["relu"], q)
                            scalar.dma_start(
                                out_d[:, stored * _P : jlo * _P],
                                out_t[:, stored * _P : jlo * _P],
                            ).then_inc(sem["outdma"], 16)
                            stored = jlo
                        if jhi + 1 == G:
                            scalar.wait_ge(sem["relu"], q + 1)
                            scalar.dma_start(
                                out_d[:, stored * _P : (jhi + 1) * _P],
                                out_t[:, stored * _P : (jhi + 1) * _P],
                            ).then_inc(sem["outdma"], 16)
                            stored = jhi + 1

            @block.vector
            def _(vector):
                vector.wait_ge(sem["cdma"], 16)
                vwaitb = [False]
                for b in range(NBLK):
                    if b >= _BLAG + 1:
                        vector.wait_ge(sem["mmblk"], b - _BLAG)
                    for c in range(b * _K, min(C, (b + 1) * _K)):
                        if c >= 96 and not vwaitb[0]:
                            vector.wait_ge(sem["cdmb"], 32)
                            vwaitb[0] = True
                        if on_dve[c]:
                            vector.tensor_scalar(
                                out=sd[ring_idx[c]][:, :],
                                in0=iota_t[:, :],
                                scalar1=dcol(c),
                                scalar2=ncol(c),
                                op0=mybir.AluOpType.is_equal,
                                op1=mybir.AluOpType.mult,
                            ).then_inc(sem["sd"], 1)

            @block.gpsimd
            def _(gpsimd):
                gpsimd.wait_ge(sem["cdma"], 16)
                pwaitb = [False]
                for b in range(NBLK):
                    if b >= _BLAG + 1:
                        gpsimd.wait_ge(sem["mmblk"], b - _BLAG)
                    for c in range(b * _K, min(C, (b + 1) * _K)):
                        if c >= 96 and not pwaitb[0]:
                            gpsimd.wait_ge(sem["cdmb"], 32)
                            pwaitb[0] = True
                        if on_pool[c]:
                            gpsimd.tensor_scalar(
                                out=sp[ring_idx[c]][:, :],
                                in0=iota_t[:, :],
                                scalar1=dcol(c),
                                scalar2=ncol(c),
                                op0=mybir.AluOpType.is_equal,
                                op1=mybir.AluOpType.mult,
                            ).then_inc(sem["sp"], 1)

            @block.tensor
            def _(tensor):
                epi = 0  # next group awaiting epilogue

                def epilogue(g):
                    # sem["aggT"]/sem["relu"] count PAIRS (2 groups each)
                    tensor.wait_ge(sem["aggT"], g // 2 + 1)
                    if g == 0:
                        tensor.wait_ge(sem["wdma"], 48)
                    if g >= 4:
                        tensor.wait_ge(sem["relu"], g // 2 - 1)
                    sl = (g % 2) * _P
                    tensor.matmul(
                        ps2[(g // 2) % 2][:, sl : sl + _P],
                        b_t[:, :],
                        p1_t[0:1, g * _P : (g + 1) * _P],
                        start=True,
                        stop=False,
                    )
                    tensor.matmul(
                        ps2[(g // 2) % 2][:, sl : sl + _P],
                        wT_t[:, :],
                        aggT[(g // 2) % 2][:, sl : sl + _P],
                        start=False,
                        stop=True,
                    ).then_inc(sem["ps2"], 1)

                for b in range(NBLK):
                    tensor.wait_ge(sem["sd"], nd_cum[b])
                    if b == 0 or np_cum[b] > np_cum[b - 1]:
                        tensor.wait_ge(sem["sp"], np_cum[b])
                    for c in range(b * _K, min(C, (b + 1) * _K)):
                        if c % _PIECE == 0:
                            tensor.wait_ge(
                                xe_sems[piece[c] % _NXB],
                                16 * (piece[c] // _NXB + 1),
                            )
                        g = grp[c]
                        if first[c] and g >= 8:
                            # ps1 slice reuse: pair copy g//2-4 must be done
                            tensor.wait_ge(sem["aggT"], g // 2 - 3)
                        S = (sd, sp)[eng_of[c]][ring_idx[c]]
                        off = (c % _PIECE) * _P
                        gsl = (g % 2) * _P
                        mm = tensor.matmul(
                            ps1[(g // 2) % 4][:, gsl : gsl + _P],
                            xep[piece[c] % _NXB][:, off : off + _P],
                            S[:, :],
                            start=bool(first[c]),
                            stop=bool(last[c]),
                        )
                        if c == blk_end[b]:
                            mm.then_inc(sem["mmblk"], 1)
                    # an epilogue is emitted 3 blocks after its PAIR's later
                    # group finishes, so the Act pair-copy it waits on has
                    # long completed and the wait never stalls the PE SEQ
                    while epi < G and grp_end_blk[min(G - 1, (epi // 2) * 2 + 1)] <= b - 3:
                        epilogue(epi)
                        epi += 1
                while epi < G:
                    epilogue(epi)
                    epi += 1

        nc.compile()
    return nc


_program_cache: dict = {}


def _get_program(cfg):
    if cfg not in _program_cache:
        _program_cache[cfg] = _build_program(cfg)
    return _program_cache[cfg]


# ---------------------------------------------------------------- entry points

def run(inputs: dict, trace: bool = False, n_cores: int = _N_CORES):
    """Run the kernel; returns (full_output, BassKernelResults)."""
    from concourse import bass_utils

    cfg, in_maps, pos_of = _host_prep(
        inputs["x"],
        inputs["W"],
        inputs["b"],
        inputs["edge_weight"],
        inputs["edge_index"],
        n_cores,
    )
    nc = _get_program(cfg)
    try:
        res = bass_utils.run_bass_kernel_spmd(
            nc, in_maps, core_ids=list(range(n_cores)), trace=trace
        )
    except Exception:
        # the axon-tunneled device occasionally reports a transient
        # NRT_EXEC_UNIT_UNRECOVERABLE right after a crashed/heavy prior run;
        # reconnect the backend and retry once before giving up
        import time as _time

        import jax as _jax

        _time.sleep(5.0)
        try:
            _jax.clear_backends()
        except Exception:
            pass
        res = bass_utils.run_bass_kernel_spmd(
            nc, in_maps, core_ids=list(range(n_cores)), trace=trace
        )
    N, nd = cfg[0], cfg[1]
    out = np.empty((N, _P), np.float32)
    for m in range(n_cores):
        slab = np.asarray(res.results[m]["outT"]).astype(np.float32).T  # [GP, 128]
        out[m * nd : (m + 1) * nd, :] = slab[pos_of[m]]
    return out, res


def kernel(**inputs) -> np.ndarray:
    out, _ = run(inputs, trace=False)
    return out
